# revision 12
# baseline (speedup 1.0000x reference)
# ARFSA attention kernel for 8 TRN2 NeuronCores (Bass/Tile).
#
# Reference computation (per batch b, channel c):
#   q = Wq x + bq ; k = Wk x + bk ; v = Wv x + bv          (1x1 convs)
#   att = softmax_flat( q @ (k + P)^T )                    (P = pos_code, same
#   out = att * v                                           for all channels,
#                                                           symmetric)
# Key tricks:
#   * Data-parallel over batch: 32 batches -> 4 per core, no collectives.
#   * P and the biases are folded into the projection matmul by augmenting
#     x with a ones-row (bias) and a P-row (added only to K channels), so
#     K+P comes straight out of PSUM.
#   * Projections run with the x-chunk as the stationary operand so tiles
#     come out transposed ([w, ch]); per-channel att matmuls then read
#     contiguous [w, 128] slices (FWL-eligible fp16 weights).
#   * Softmax without max-subtraction (logits are bounded ~|45|, safe in
#     fp32 exp / bf16 storage).
#   * fp16 inputs / bf16 outputs halve HBM traffic and PSUM->SBUF copies
#     are the bottleneck; they are split between ScalarE and VectorE.
#
# Layouts (per core):
#   xa   DRAM in  [4, 66, 16384] fp16   rows 0..63 = x, row 64 = 1.0 (bias),
#                                       row 65 = P.flatten() (K-only via waug)
#   waug DRAM in  [66, 192] fp16        cols 0:64 Wq^T | 64:128 Wk^T | 128:192 Wv^T
#   out  DRAM out [4, 128(w), 64(c), 128(h)] bf16   (host transposes to [b,c,h,w])

import sys
import os

if "/opt/trn_rl_repo" not in sys.path:
    sys.path.insert(0, "/opt/trn_rl_repo")

import numpy as np
from contextlib import ExitStack

import concourse.bass as bass
import concourse.tile as tile
from concourse import bacc, mybir
from concourse.bass_utils import run_bass_kernel_spmd

N_CORES = 8
B_LOC = 4            # 32 batches / 8 cores
C = 64               # out channels
F = 128              # feature map size
S = F * F            # 16384 positions

FP16 = mybir.dt.float16
BF16 = mybir.dt.bfloat16
F32 = mybir.dt.float32

_BUILT = {}

# Engine assignment knobs (tuned from traces).
QK_COPY_ACT_EVERY = 1   # every Nth 4j-group's QK copy goes to ScalarE
V_COPY_ACT_EVERY = 0    # 0 = never on ScalarE (VectorE)


def _build_bass():
    nc = bacc.Bacc("TRN2", target_bir_lowering=False, debug=False)

    xa = nc.declare_dram_parameter("xa", [B_LOC, 66, S], FP16, isOutput=False)
    waug = nc.declare_dram_parameter("waug", [66, 192], FP16, isOutput=False)
    out = nc.declare_dram_parameter("out", [B_LOC, F, C, F], BF16, isOutput=True)

    with ExitStack() as ctx:
        tc = ctx.enter_context(tile.TileContext(nc))

        const = ctx.enter_context(tc.tile_pool(name="const", bufs=1))
        xpool = ctx.enter_context(tc.tile_pool(name="xpool", bufs=3))
        qkpool = ctx.enter_context(tc.tile_pool(name="qkpool", bufs=2))
        kcmpool = ctx.enter_context(tc.tile_pool(name="kcmpool", bufs=2))
        vpool = ctx.enter_context(tc.tile_pool(name="vpool", bufs=2))
        epool = ctx.enter_context(tc.tile_pool(name="epool", bufs=2))
        rpool = ctx.enter_context(tc.tile_pool(name="rpool", bufs=2))
        opool = ctx.enter_context(tc.tile_pool(name="opool", bufs=3))
        proj_ps = ctx.enter_context(tc.tile_pool(name="proj_ps", bufs=2, space="PSUM"))
        att_ps = ctx.enter_context(tc.tile_pool(name="att_ps", bufs=2, space="PSUM"))

        waug_sb = const.tile([66, 192], FP16, tag="waug")
        nc.sync.dma_start(out=waug_sb[:], in_=waug[:, :])
        ones_sb = const.tile([128, 128], F32, tag="ones")
        nc.vector.memset(ones_sb[:], 1.0)

        for b in range(B_LOC):
            # ---- stream x in 2048-column chunks (16 j-tiles each) ----
            x_tiles = []
            for xc in range(8):
                x_t = xpool.tile([66, 2048], FP16, tag="xt")
                nc.sync.dma_start(out=x_t[:], in_=xa[b, :, xc * 2048:(xc + 1) * 2048])
                x_tiles.append(x_t)

            # ---- projections: 128 j-tiles, groups of 4 per PSUM tile ----
            # psum slot stride padded to 256 f32 so each matmul output stays
            # inside one 2KB bank.
            # j-major layouts: PSUM->SBUF copies land contiguous (the att
            # matmuls read strided [.., .., ch] slices instead, which the
            # PE streams at full rate anyway).
            qk_sb = qkpool.tile([128, F, 128], FP16, tag="qk")   # [w, j, q0..63|k0..63]
            v_sb = vpool.tile([128, F, C], BF16, tag="v")        # [w, j, c]
            for g in range(32):
                pt = proj_ps.tile([128, 4, 256], F32, tag="proj")
                for jj in range(4):
                    j = g * 4 + jj
                    x_t = x_tiles[j // 16]
                    nc.tensor.matmul(
                        pt[:, jj, 0:192],
                        lhsT=x_t[:, (j % 16) * F:(j % 16 + 1) * F],
                        rhs=waug_sb[:],
                        start=True, stop=True,
                    )
                qk_eng = nc.scalar if (QK_COPY_ACT_EVERY and g % QK_COPY_ACT_EVERY == 0) else nc.vector
                v_eng = nc.scalar if (V_COPY_ACT_EVERY and g % V_COPY_ACT_EVERY == 0) else nc.vector
                if qk_eng is nc.scalar:
                    nc.scalar.copy(qk_sb[:, g * 4:(g + 1) * 4, :], pt[:, :, 0:128])
                else:
                    nc.vector.tensor_copy(qk_sb[:, g * 4:(g + 1) * 4, :], pt[:, :, 0:128])
                if v_eng is nc.scalar:
                    nc.scalar.copy(v_sb[:, g * 4:(g + 1) * 4, :], pt[:, :, 128:192])
                else:
                    nc.vector.tensor_copy(v_sb[:, g * 4:(g + 1) * 4, :], pt[:, :, 128:192])

            # ---- re-layout K+P to c-major (contiguous LDWEIGHTS) on the
            # otherwise-idle GpSimd engine ----
            k_cm = kcmpool.tile([128, C, F], FP16, tag="kcm")    # [w, c, v]
            qk_cm = qk_sb.rearrange("p j c -> p c j")
            for rg in range(8):
                nc.gpsimd.tensor_copy(k_cm[:, rg * 8:(rg + 1) * 8, :],
                                      qk_cm[:, 64 + rg * 8:64 + (rg + 1) * 8, :])

            # ---- attention: per channel matmul + batched exp/rowsum ----
            e_sb = epool.tile([128, C, F], BF16, tag="e")        # [v, c, h]
            r_all = rpool.tile([128, C], F32, tag="r")
            for cg in range(8):
                at = att_ps.tile([128, 8, F], F32, tag="att")
                for cc in range(8):
                    c = cg * 8 + cc
                    nc.tensor.matmul(
                        at[:, cc, :],
                        lhsT=k_cm[:, c, :],         # (K+P)^T tile [w, v]
                        rhs=qk_sb[:, :, c],         # Q^T tile [w, h]
                        start=True, stop=True,
                    )
                nc.scalar.activation(
                    e_sb[:, cg * 8:(cg + 1) * 8, :], at[:, :, :],
                    mybir.ActivationFunctionType.Exp,
                )
                nc.vector.tensor_reduce(
                    r_all[:, cg * 8:(cg + 1) * 8], e_sb[:, cg * 8:(cg + 1) * 8, :],
                    axis=mybir.AxisListType.X, op=mybir.AluOpType.add,
                )

            # ---- softmax denominators: cross-partition sum via ones-matmul ----
            sp = att_ps.tile([128, C], F32, tag="att")
            nc.tensor.matmul(sp[:, :], lhsT=ones_sb[:], rhs=r_all[:, :],
                             start=True, stop=True)
            sinv = rpool.tile([128, C], F32, tag="sinv")
            nc.vector.reciprocal(sinv[:], sp[:, :])

            # ---- out = (E * sinv) * V, 8 channels per output DMA ----
            # Per-channel scale runs in DVE 4x mode (contiguous bf16
            # tensor_scalar); the E*V product is a batched tensor_tensor.
            for c in range(C):
                nc.vector.tensor_scalar_mul(e_sb[:, c, :], e_sb[:, c, :],
                                            sinv[:, c:c + 1])
            v_cm = v_sb.rearrange("p j c -> p c j")
            for og in range(8):
                ot = opool.tile([128, 8, F], BF16, tag="ot")
                for q in range(2):
                    c0 = og * 8 + q * 4
                    nc.gpsimd.tensor_mul(
                        ot[:, q * 4:(q + 1) * 4, :],
                        e_sb[:, c0:c0 + 4, :],
                        v_cm[:, c0:c0 + 4, :],
                    )
                nc.sync.dma_start(out=out[b, :, og * 8:(og + 1) * 8, :], in_=ot[:])

    nc.compile()
    return nc


def _get_built():
    if "nc" not in _BUILT:
        _BUILT["nc"] = _build_bass()
    return _BUILT["nc"]


def _prep_inputs(x, wq, bq, wk, bk, wv, bv, pos_code):
    x = np.asarray(x, np.float32)
    pos = np.asarray(pos_code, np.float32)[0]          # identical across channels
    waug = np.zeros([66, 192], np.float32)
    waug[0:64, 0:64] = np.asarray(wq, np.float32).T
    waug[0:64, 64:128] = np.asarray(wk, np.float32).T
    waug[0:64, 128:192] = np.asarray(wv, np.float32).T
    waug[64, 0:64] = np.asarray(bq, np.float32)
    waug[64, 64:128] = np.asarray(bk, np.float32)
    waug[64, 128:192] = np.asarray(bv, np.float32)
    waug[65, 64:128] = 1.0                             # P-row hits K channels only
    waug16 = waug.astype(np.float16)

    pflat16 = pos.reshape(-1).astype(np.float16)
    xf = x.reshape(x.shape[0], x.shape[1], S)
    in_maps = []
    for core in range(N_CORES):
        xs = xf[core * B_LOC:(core + 1) * B_LOC]
        xa = np.empty([B_LOC, 66, S], np.float16)
        xa[:, 0:64] = xs.astype(np.float16)
        xa[:, 64] = np.float16(1.0)
        xa[:, 65] = pflat16[None, :]
        in_maps.append({"xa": xa, "waug": waug16})
    return in_maps


LAST_RESULTS = None


def kernel(x, wq, bq, wk, bk, wv, bv, pos_code, _trace=False):
    global LAST_RESULTS
    in_maps = _prep_inputs(x, wq, bq, wk, bk, wv, bv, pos_code)
    nc = _get_built()
    res = run_bass_kernel_spmd(nc, in_maps, core_ids=list(range(N_CORES)),
                               trace=_trace)
    LAST_RESULTS = res
    outs = []
    for core in range(N_CORES):
        o = np.asarray(res.results[core]["out"])       # [4, w, c, h] bf16
        outs.append(np.transpose(o.astype(np.float32), (0, 2, 3, 1)))
    return np.concatenate(outs, axis=0)


# revision 17
# speedup vs baseline: 1.1381x; 1.1381x over previous
# ARFSA attention kernel for 8 TRN2 NeuronCores (Bass/Tile).
#
# Reference computation (per batch b, channel c):
#   q = Wq x + bq ; k = Wk x + bk ; v = Wv x + bv          (1x1 convs)
#   att = softmax_flat( q @ (k + P)^T )                    (P = pos_code, same
#   out = att * v                                           for all channels,
#                                                           symmetric)
# Key tricks:
#   * Data-parallel over batch: 32 batches -> 4 per core, no collectives.
#   * P and the biases are folded into the projection matmul by augmenting
#     x with a ones-row (bias) and a P-row (added only to K channels), so
#     K+P comes straight out of PSUM.
#   * Projections run with the x-chunk as the stationary operand so tiles
#     come out transposed ([w, ch]); per-channel att matmuls then read
#     contiguous [w, 128] slices (FWL-eligible fp16 weights).
#   * Softmax without max-subtraction (logits are bounded ~|45|, safe in
#     fp32 exp / bf16 storage).
#   * fp16 inputs / bf16 outputs halve HBM traffic and PSUM->SBUF copies
#     are the bottleneck; they are split between ScalarE and VectorE.
#
# Layouts (per core):
#   xa   DRAM in  [4, 66, 16384] fp16   rows 0..63 = x, row 64 = 1.0 (bias),
#                                       row 65 = P.flatten() (K-only via waug)
#   waug DRAM in  [66, 192] fp16        cols 0:64 Wq^T | 64:128 Wk^T | 128:192 Wv^T
#   out  DRAM out [4, 128(w), 64(c), 128(h)] bf16   (host transposes to [b,c,h,w])

import sys
import os

if "/opt/trn_rl_repo" not in sys.path:
    sys.path.insert(0, "/opt/trn_rl_repo")

import numpy as np
from contextlib import ExitStack

import concourse.bass as bass
import concourse.tile as tile
from concourse import bacc, mybir
from concourse.bass_utils import run_bass_kernel_spmd

N_CORES = 8
B_LOC = 4            # 32 batches / 8 cores
C = 64               # out channels
F = 128              # feature map size
S = F * F            # 16384 positions

FP16 = mybir.dt.float16
BF16 = mybir.dt.bfloat16
F32 = mybir.dt.float32

_BUILT = {}

# Engine assignment knobs (tuned from traces).
QK_COPY_ACT_EVERY = 1   # every Nth 4j-group's QK copy goes to ScalarE
V_COPY_ACT_EVERY = 3    # every Nth 4j-group's V copy goes to ScalarE


def _build_bass():
    nc = bacc.Bacc("TRN2", target_bir_lowering=False, debug=False)

    xa = nc.declare_dram_parameter("xa", [B_LOC, 66, S], FP16, isOutput=False)
    waug = nc.declare_dram_parameter("waug", [66, 192], FP16, isOutput=False)
    out = nc.declare_dram_parameter("out", [B_LOC, F, C, F], BF16, isOutput=True)

    with ExitStack() as ctx:
        tc = ctx.enter_context(tile.TileContext(nc))

        const = ctx.enter_context(tc.tile_pool(name="const", bufs=1))
        xpool = ctx.enter_context(tc.tile_pool(name="xpool", bufs=3))
        qkpool = ctx.enter_context(tc.tile_pool(name="qkpool", bufs=2))
        vpool = ctx.enter_context(tc.tile_pool(name="vpool", bufs=2))
        epool = ctx.enter_context(tc.tile_pool(name="epool", bufs=2))
        rpool = ctx.enter_context(tc.tile_pool(name="rpool", bufs=2))
        opool = ctx.enter_context(tc.tile_pool(name="opool", bufs=3))
        proj_ps = ctx.enter_context(tc.tile_pool(name="proj_ps", bufs=3, space="PSUM"))
        att_ps = ctx.enter_context(tc.tile_pool(name="att_ps", bufs=2, space="PSUM"))

        waug_sb = const.tile([66, 192], FP16, tag="waug")
        nc.sync.dma_start(out=waug_sb[:], in_=waug[:, :])
        ones_sb = const.tile([128, 128], F32, tag="ones")
        nc.vector.memset(ones_sb[:], 1.0)

        for b in range(B_LOC):
            # ---- stream x in 2048-column chunks (16 j-tiles each) ----
            x_tiles = []
            for xc in range(8):
                x_t = xpool.tile([66, 2048], FP16, tag="xt")
                nc.sync.dma_start(out=x_t[:], in_=xa[b, :, xc * 2048:(xc + 1) * 2048])
                x_tiles.append(x_t)

            # ---- projections: 128 j-tiles, groups of 4 per PSUM tile ----
            # psum slot stride padded to 256 f32 so each matmul output stays
            # inside one 2KB bank.
            # j-major layouts: PSUM->SBUF copies land contiguous (the att
            # matmuls read strided [.., .., ch] slices instead, which the
            # PE streams at full rate anyway).
            qk_sb = qkpool.tile([128, F, 128], FP16, tag="qk")   # [w, j, q0..63|k0..63]
            v_sb = vpool.tile([128, F, C], BF16, tag="v")        # [w, j, c]
            for g in range(32):
                pt = proj_ps.tile([128, 4, 256], F32, tag="proj")
                for jj in range(4):
                    j = g * 4 + jj
                    x_t = x_tiles[j // 16]
                    nc.tensor.matmul(
                        pt[:, jj, 0:192],
                        lhsT=x_t[:, (j % 16) * F:(j % 16 + 1) * F],
                        rhs=waug_sb[:],
                        start=True, stop=True,
                    )
                qk_eng = nc.scalar if (QK_COPY_ACT_EVERY and g % QK_COPY_ACT_EVERY == 0) else nc.vector
                v_eng = nc.scalar if (V_COPY_ACT_EVERY and g % V_COPY_ACT_EVERY == 0) else nc.vector
                if qk_eng is nc.scalar:
                    nc.scalar.copy(qk_sb[:, g * 4:(g + 1) * 4, :], pt[:, :, 0:128])
                else:
                    nc.vector.tensor_copy(qk_sb[:, g * 4:(g + 1) * 4, :], pt[:, :, 0:128])
                if v_eng is nc.scalar:
                    nc.scalar.copy(v_sb[:, g * 4:(g + 1) * 4, :], pt[:, :, 128:192])
                else:
                    nc.vector.tensor_copy(v_sb[:, g * 4:(g + 1) * 4, :], pt[:, :, 128:192])

            # ---- attention: per channel matmul + batched exp/rowsum ----
            e_sb = epool.tile([128, C, F], BF16, tag="e")        # [v, c, h]
            r_all = rpool.tile([128, C], F32, tag="r")
            for cg in range(16):
                at = att_ps.tile([128, 4, F], F32, tag="att")
                for cc in range(4):
                    c = cg * 4 + cc
                    nc.tensor.matmul(
                        at[:, cc, :],
                        lhsT=qk_sb[:, :, 64 + c],   # (K+P)^T tile [w, v]
                        rhs=qk_sb[:, :, c],         # Q^T tile [w, h]
                        start=True, stop=True,
                    )
                nc.scalar.activation(
                    e_sb[:, cg * 4:(cg + 1) * 4, :], at[:, :, :],
                    mybir.ActivationFunctionType.Exp,
                )
                nc.vector.tensor_reduce(
                    r_all[:, cg * 4:(cg + 1) * 4], e_sb[:, cg * 4:(cg + 1) * 4, :],
                    axis=mybir.AxisListType.X, op=mybir.AluOpType.add,
                )

            # ---- softmax denominators: cross-partition sum via ones-matmul ----
            sp = att_ps.tile([128, C], F32, tag="att")
            nc.tensor.matmul(sp[:, :], lhsT=ones_sb[:], rhs=r_all[:, :],
                             start=True, stop=True)
            sinv = rpool.tile([128, C], F32, tag="sinv")
            nc.vector.reciprocal(sinv[:], sp[:, :])

            # ---- out = (E * sinv) * V, 8 channels per output DMA ----
            # Per-channel scale runs in DVE 4x mode (contiguous bf16
            # tensor_scalar); the E*V product is a batched tensor_tensor.
            for c in range(C):
                nc.vector.tensor_scalar_mul(e_sb[:, c, :], e_sb[:, c, :],
                                            sinv[:, c:c + 1])
            v_cm = v_sb.rearrange("p j c -> p c j")
            for og in range(8):
                ot = opool.tile([128, 8, F], BF16, tag="ot")
                for q in range(2):
                    c0 = og * 8 + q * 4
                    nc.vector.tensor_mul(
                        ot[:, q * 4:(q + 1) * 4, :],
                        e_sb[:, c0:c0 + 4, :],
                        v_cm[:, c0:c0 + 4, :],
                    )
                nc.sync.dma_start(out=out[b, :, og * 8:(og + 1) * 8, :], in_=ot[:])

    nc.compile()
    return nc


def _get_built():
    if "nc" not in _BUILT:
        _BUILT["nc"] = _build_bass()
    return _BUILT["nc"]


def _prep_inputs(x, wq, bq, wk, bk, wv, bv, pos_code):
    x = np.asarray(x, np.float32)
    pos = np.asarray(pos_code, np.float32)[0]          # identical across channels
    waug = np.zeros([66, 192], np.float32)
    waug[0:64, 0:64] = np.asarray(wq, np.float32).T
    waug[0:64, 64:128] = np.asarray(wk, np.float32).T
    waug[0:64, 128:192] = np.asarray(wv, np.float32).T
    waug[64, 0:64] = np.asarray(bq, np.float32)
    waug[64, 64:128] = np.asarray(bk, np.float32)
    waug[64, 128:192] = np.asarray(bv, np.float32)
    waug[65, 64:128] = 1.0                             # P-row hits K channels only
    waug16 = waug.astype(np.float16)

    pflat16 = pos.reshape(-1).astype(np.float16)
    xf = x.reshape(x.shape[0], x.shape[1], S)
    in_maps = []
    for core in range(N_CORES):
        xs = xf[core * B_LOC:(core + 1) * B_LOC]
        xa = np.empty([B_LOC, 66, S], np.float16)
        xa[:, 0:64] = xs.astype(np.float16)
        xa[:, 64] = np.float16(1.0)
        xa[:, 65] = pflat16[None, :]
        in_maps.append({"xa": xa, "waug": waug16})
    return in_maps


LAST_RESULTS = None


def kernel(x, wq, bq, wk, bk, wv, bv, pos_code, _trace=False):
    global LAST_RESULTS
    in_maps = _prep_inputs(x, wq, bq, wk, bk, wv, bv, pos_code)
    nc = _get_built()
    res = run_bass_kernel_spmd(nc, in_maps, core_ids=list(range(N_CORES)),
                               trace=_trace)
    LAST_RESULTS = res
    outs = []
    for core in range(N_CORES):
        o = np.asarray(res.results[core]["out"])       # [4, w, c, h] bf16
        outs.append(np.transpose(o.astype(np.float32), (0, 2, 3, 1)))
    return np.concatenate(outs, axis=0)


# revision 20
# speedup vs baseline: 1.5167x; 1.3326x over previous
# ARFSA attention kernel for 8 TRN2 NeuronCores (Bass/Tile).
#
# Reference computation (per batch b, channel c):
#   q = Wq x + bq ; k = Wk x + bk ; v = Wv x + bv          (1x1 convs)
#   att = softmax_flat( q @ (k + P)^T )                    (P = pos_code, same
#   out = att * v                                           for all channels,
#                                                           symmetric)
# Key tricks:
#   * Data-parallel over batch: 32 batches -> 4 per core, no collectives.
#   * P and the biases are folded into the projection matmul by augmenting
#     x with a ones-row (bias) and a P-row (added only to K channels), so
#     K+P comes straight out of PSUM.
#   * Projections run with the x-chunk as the stationary operand so tiles
#     come out transposed ([w, ch]); per-channel att matmuls then read
#     contiguous [w, 128] slices (FWL-eligible fp16 weights).
#   * Softmax without max-subtraction (logits are bounded ~|45|, safe in
#     fp32 exp / bf16 storage).
#   * fp16 inputs / bf16 outputs halve HBM traffic and PSUM->SBUF copies
#     are the bottleneck; they are split between ScalarE and VectorE.
#
# Layouts (per core):
#   xa   DRAM in  [4, 66, 16384] fp16   rows 0..63 = x, row 64 = 1.0 (bias),
#                                       row 65 = P.flatten() (K-only via waug)
#   waug DRAM in  [66, 192] fp16        cols 0:64 Wq^T | 64:128 Wk^T | 128:192 Wv^T
#   out  DRAM out [4, 128(w), 64(c), 128(h)] bf16   (host transposes to [b,c,h,w])

import sys
import os

if "/opt/trn_rl_repo" not in sys.path:
    sys.path.insert(0, "/opt/trn_rl_repo")

import numpy as np
from contextlib import ExitStack

import concourse.bass as bass
import concourse.tile as tile
from concourse import bacc, mybir
from concourse.bass_utils import run_bass_kernel_spmd

N_CORES = 8
B_LOC = 4            # 32 batches / 8 cores
C = 64               # out channels
F = 128              # feature map size
S = F * F            # 16384 positions

FP16 = mybir.dt.float16
BF16 = mybir.dt.bfloat16
F32 = mybir.dt.float32

_BUILT = {}

# Engine assignment knobs (tuned from traces).
QK_COPY_ACT_EVERY = 1   # every Nth 4j-group's QK copy goes to ScalarE
V_COPY_ACT_EVERY = 3    # every Nth 4j-group's V copy goes to ScalarE


def _build_bass():
    nc = bacc.Bacc("TRN2", target_bir_lowering=False, debug=False)

    xa = nc.declare_dram_parameter("xa", [B_LOC, 66, S], FP16, isOutput=False)
    waug = nc.declare_dram_parameter("waug", [66, 192], FP16, isOutput=False)
    out = nc.declare_dram_parameter("out", [B_LOC, F, C, F], BF16, isOutput=True)

    with ExitStack() as ctx:
        tc = ctx.enter_context(tile.TileContext(nc))

        const = ctx.enter_context(tc.tile_pool(name="const", bufs=1))
        xpool = ctx.enter_context(tc.tile_pool(name="xpool", bufs=3))
        qkpool = ctx.enter_context(tc.tile_pool(name="qkpool", bufs=2))
        vpool = ctx.enter_context(tc.tile_pool(name="vpool", bufs=2))
        epool = ctx.enter_context(tc.tile_pool(name="epool", bufs=2))
        rpool = ctx.enter_context(tc.tile_pool(name="rpool", bufs=2))
        opool = ctx.enter_context(tc.tile_pool(name="opool", bufs=3))
        ps = ctx.enter_context(tc.tile_pool(name="ps", bufs=2, space="PSUM"))

        waug_sb = const.tile([66, 192], FP16, tag="waug")
        nc.sync.dma_start(out=waug_sb[:], in_=waug[:, :])
        ones_sb = const.tile([128, 128], F32, tag="ones")
        nc.vector.memset(ones_sb[:], 1.0)

        # Per-b state carried between loop iterations so b's pass2 can be
        # emitted interleaved with b+1's projection phase (keeps VectorE's
        # end-of-batch chain from stalling the PE at batch boundaries).
        state = {}

        def emit_pass2_chunk(st, og):
            b, e_sb, v_cm = st["b"], st["e"], st["v_cm"]
            ot = opool.tile([128, 8, F], BF16, tag="ot", name=f"ot_{b}_{og}")
            for q in range(2):
                c0 = og * 8 + q * 4
                nc.vector.tensor_mul(
                    ot[:, q * 4:(q + 1) * 4, :],
                    e_sb[:, c0:c0 + 4, :],
                    v_cm[:, c0:c0 + 4, :],
                )
            nc.sync.dma_start(out=out[b, :, og * 8:(og + 1) * 8, :], in_=ot[:])

        for b in range(B_LOC + 1):
            if b < B_LOC:
                x_tiles = []
                for xc in range(8):
                    x_t = xpool.tile([66, 2048], FP16, tag="xt",
                                     name=f"xt_{b}_{xc}")
                    nc.sync.dma_start(out=x_t[:],
                                      in_=xa[b, :, xc * 2048:(xc + 1) * 2048])
                    x_tiles.append(x_t)
                qk_sb = qkpool.tile([128, F, 128], FP16, tag="qk",
                                    name=f"qk_{b}")   # [w, j, q0..63|k0..63]
                v_sb = vpool.tile([128, F, C], BF16, tag="v", name=f"v_{b}")

            # ---- projections (8 j per 4-bank PSUM tile) interleaved with
            # previous batch's pass2 ----
            for g in range(16):
                if b < B_LOC:
                    pt = ps.tile([128, 8, 256], F32, tag="ps",
                                 name=f"pt_{b}_{g}")
                    for jj in range(8):
                        j = g * 8 + jj
                        x_t = x_tiles[j // 16]
                        nc.tensor.matmul(
                            pt[:, jj, 0:192],
                            lhsT=x_t[:, (j % 16) * F:(j % 16 + 1) * F],
                            rhs=waug_sb[:],
                            start=True, stop=True,
                        )
                    nc.scalar.copy(qk_sb[:, g * 8:(g + 1) * 8, :],
                                   pt[:, :, 0:128])
                    nc.scalar.copy(v_sb[:, g * 8:(g + 1) * 8, :],
                                   pt[:, :, 128:192])
                if b >= 1 and g % 2 == 0:
                    emit_pass2_chunk(state, g // 2)

            if b >= B_LOC:
                break

            # ---- attention: per channel matmul + batched exp/rowsum ----
            e_sb = epool.tile([128, C, F], BF16, tag="e", name=f"e_{b}")
            r_all = rpool.tile([128, C], F32, tag="r", name=f"r_{b}")
            for cg in range(8):
                at = ps.tile([128, 8, F], F32, tag="ps", name=f"at_{b}_{cg}")
                for cc in range(8):
                    c = cg * 8 + cc
                    nc.tensor.matmul(
                        at[:, cc, :],
                        lhsT=qk_sb[:, :, 64 + c],   # (K+P)^T tile [w, v]
                        rhs=qk_sb[:, :, c],         # Q^T tile [w, h]
                        start=True, stop=True,
                    )
                nc.scalar.activation(
                    e_sb[:, cg * 8:(cg + 1) * 8, :], at[:, :, :],
                    mybir.ActivationFunctionType.Exp,
                )
                nc.vector.tensor_reduce(
                    r_all[:, cg * 8:(cg + 1) * 8], e_sb[:, cg * 8:(cg + 1) * 8, :],
                    axis=mybir.AxisListType.X, op=mybir.AluOpType.add,
                )

            # ---- softmax denominators: cross-partition sum via ones-matmul ----
            sp = ps.tile([128, C], F32, tag="ps", name=f"sp_{b}")
            nc.tensor.matmul(sp[:, :], lhsT=ones_sb[:], rhs=r_all[:, :],
                             start=True, stop=True)
            sinv = rpool.tile([128, C], F32, tag="sinv", name=f"sinv_{b}")
            nc.vector.reciprocal(sinv[:], sp[:, :])

            # ---- fold 1/S into E (DVE 4x-mode per-channel tensor_scalar) ----
            for c in range(C):
                nc.vector.tensor_scalar_mul(e_sb[:, c, :], e_sb[:, c, :],
                                            sinv[:, c:c + 1])
            state = {"b": b, "e": e_sb, "v_cm": v_sb.rearrange("p j c -> p c j")}

    nc.compile()
    return nc


def _get_built():
    if "nc" not in _BUILT:
        _BUILT["nc"] = _build_bass()
    return _BUILT["nc"]


def _prep_inputs(x, wq, bq, wk, bk, wv, bv, pos_code):
    x = np.asarray(x, np.float32)
    pos = np.asarray(pos_code, np.float32)[0]          # identical across channels
    waug = np.zeros([66, 192], np.float32)
    waug[0:64, 0:64] = np.asarray(wq, np.float32).T
    waug[0:64, 64:128] = np.asarray(wk, np.float32).T
    waug[0:64, 128:192] = np.asarray(wv, np.float32).T
    waug[64, 0:64] = np.asarray(bq, np.float32)
    waug[64, 64:128] = np.asarray(bk, np.float32)
    waug[64, 128:192] = np.asarray(bv, np.float32)
    waug[65, 64:128] = 1.0                             # P-row hits K channels only
    waug16 = waug.astype(np.float16)

    pflat16 = pos.reshape(-1).astype(np.float16)
    xf = x.reshape(x.shape[0], x.shape[1], S)
    in_maps = []
    for core in range(N_CORES):
        xs = xf[core * B_LOC:(core + 1) * B_LOC]
        xa = np.empty([B_LOC, 66, S], np.float16)
        xa[:, 0:64] = xs.astype(np.float16)
        xa[:, 64] = np.float16(1.0)
        xa[:, 65] = pflat16[None, :]
        in_maps.append({"xa": xa, "waug": waug16})
    return in_maps


LAST_RESULTS = None


def kernel(x, wq, bq, wk, bk, wv, bv, pos_code, _trace=False):
    global LAST_RESULTS
    in_maps = _prep_inputs(x, wq, bq, wk, bk, wv, bv, pos_code)
    nc = _get_built()
    res = run_bass_kernel_spmd(nc, in_maps, core_ids=list(range(N_CORES)),
                               trace=_trace)
    LAST_RESULTS = res
    outs = []
    for core in range(N_CORES):
        o = np.asarray(res.results[core]["out"])       # [4, w, c, h] bf16
        outs.append(np.transpose(o.astype(np.float32), (0, 2, 3, 1)))
    return np.concatenate(outs, axis=0)
